# revision 6
# baseline (speedup 1.0000x reference)
"""GATv2Conv forward on 8 Trainium2 NeuronCores (Bass/Tile).

Strategy
--------
Edges are sorted by destination node and packed into "groups" of at most
S=256 edge slots / at most 128 distinct destinations, such that every
destination's edge run lies entirely inside one group.  Groups are split
evenly across the 8 cores, so all softmax segments and output rows are
core-local: no collectives are needed.

Per core (SPMD, identical program; per-core data differs):
  - el_e = feat[src]@W_src, er_e = feat[dst]@W_dst as dense fp16 GEMMs over
    host-pre-gathered edge features (128-edge chunks, K=256 split in two).
  - score = (prelu(el+er, 0.2) * attn).sum(-1)  (ACT Prelu + mult + reduce)
  - ex = exp(score)  (batched over 8 chunks per ACT op)
  - per group: psum[rank, :] += Hx^T @ [1 | el]  where Hx[e,k] =
    (rank_e==k)*ex_e  (one-hot matmul scatter-add; ssum lands in col 0)
  - out rows = psum[:,1:257] * 1/max(ssum,tiny)  -> dense per-group rows.

Host unshards by scattering dense group rows to their global node ids
(pure index plumbing; all arithmetic happens on-device).
"""

import math
import numpy as np

import concourse.bass as bass
import concourse.mybir as mybir
import concourse.tile as tile
from concourse import bacc
from concourse.bass_utils import run_bass_kernel_spmd

F32 = mybir.dt.float32
F16 = mybir.dt.float16
BF16 = mybir.dt.bfloat16
I32 = mybir.dt.int32
AF = mybir.ActivationFunctionType
ALU = mybir.AluOpType
AX = mybir.AxisListType

N_CORES = 8
S = 256            # edge slots per group (2 chunks of 128)
CHUNK = 128
GPB = 4            # groups per batch (exp batched over 2*GPB chunks)
CPB = 2 * GPB      # chunks per batch
SC = S * GPB       # edge slots per batch
NEG_SLOPE = 0.2

LAST_RESULTS = None  # BassKernelResults of the most recent run (for test.py)
LAST_NC = None       # compiled Bacc program of the most recent run
LAST_IN_MAPS = None  # per-core input dicts of the most recent run


# ----------------------------------------------------------------- host prep

def _pack_runs(counts, max_slots=S, max_nodes=128):
    """Greedily pack whole runs (same-dst edge blocks) into groups."""
    n = len(counts)
    grp = np.empty(n, np.int64)
    rank = np.empty(n, np.int64)
    g = used = nodes = 0
    for i in range(n):
        c = counts[i]
        if used + c > max_slots or nodes >= max_nodes:
            g += 1
            used = 0
            nodes = 0
        grp[i] = g
        rank[i] = nodes
        used += c
        nodes += 1
    return grp, rank, g + 1


def _prepare(feat, src, dst):
    """Sort edges by dst, pack into groups, build per-core input arrays."""
    E = dst.shape[0]
    order = np.argsort(dst, kind="stable")
    sd = dst[order].astype(np.int64)
    ss = src[order].astype(np.int64)

    uniq, counts = np.unique(sd, return_counts=True)
    # split pathological runs longer than S so packing can't fail
    need_accum = bool((counts > S).any())
    if need_accum:
        new_uniq, new_counts = [], []
        for u, c in zip(uniq, counts):
            while c > S:
                new_uniq.append(u)
                new_counts.append(S)
                c -= S
            new_uniq.append(u)
            new_counts.append(c)
        uniq = np.array(new_uniq, np.int64)
        counts = np.array(new_counts, np.int64)

    grp, rank, g_tot = _pack_runs(counts)
    n_runs = len(counts)

    starts = np.zeros(n_runs, np.int64)
    np.cumsum(counts[:-1], out=starts[1:])
    grp_first_run = np.searchsorted(grp, np.arange(g_tot))
    grp_start_edge = starts[grp_first_run]

    run_of_edge = np.repeat(np.arange(n_runs), counts)
    e_grp = grp[run_of_edge]
    e_rank = rank[run_of_edge]
    e_slot = e_grp * S + np.arange(E) - grp_start_edge[e_grp]

    g_pc = math.ceil(g_tot / N_CORES)
    g_pc = math.ceil(g_pc / GPB) * GPB          # multiple of GPB per core
    e_slots = g_pc * S
    total_slots = N_CORES * e_slots

    slot_src = np.zeros(total_slots, np.int64)
    slot_dst = np.zeros(total_slots, np.int64)
    slot_rank = np.full(total_slots, -1.0, np.float32)
    slot_src[e_slot] = ss
    slot_dst[e_slot] = sd
    slot_rank[e_slot] = e_rank

    # unshard info per run
    run_core = grp // g_pc
    run_pos = (grp % g_pc) * 128 + rank        # row in the core's dense output

    return dict(
        g_pc=g_pc, e_slots=e_slots, need_accum=need_accum,
        slot_src=slot_src, slot_dst=slot_dst, slot_rank=slot_rank,
        run_core=run_core, run_pos=run_pos, run_node=uniq,
    )


# ------------------------------------------------------------ device program

def _build_program(n_g, has_bias):
    nc_chunks = n_g * 2
    e_slots = n_g * S
    nb = n_g // GPB
    nc = bacc.Bacc("TRN2", target_bir_lowering=False, debug=False,
                   num_devices=N_CORES)
    fsT_d = nc.dram_tensor("fsT", [2, 128, e_slots], F16, kind="ExternalInput").ap()
    fdT_d = nc.dram_tensor("fdT", [2, 128, e_slots], F16, kind="ExternalInput").ap()
    rankT_d = nc.dram_tensor("rankT", [128, nc_chunks], F32, kind="ExternalInput").ap()
    wsrc_d = nc.dram_tensor("wsrc", [2, 128, 256], F16, kind="ExternalInput").ap()
    wdst_d = nc.dram_tensor("wdst", [2, 128, 256], F16, kind="ExternalInput").ap()
    attn_d = nc.dram_tensor("attn", [1, 256], F32, kind="ExternalInput").ap()
    if has_bias:
        bsrc_d = nc.dram_tensor("bsrc", [1, 256], F16, kind="ExternalInput").ap()
        bdst_d = nc.dram_tensor("bdst", [1, 256], F16, kind="ExternalInput").ap()
    dense_d = nc.dram_tensor("dense", [n_g * 128, 256], F32,
                             kind="ExternalOutput").ap()

    with tile.TileContext(nc) as tc:
        with (
            tc.tile_pool(name="const", bufs=1) as cpool,
            tc.tile_pool(name="fsp", bufs=2) as fs_pool,
            tc.tile_pool(name="fdp", bufs=2) as fd_pool,
            tc.tile_pool(name="rkp", bufs=2) as rk_pool,
            tc.tile_pool(name="scp", bufs=2) as sc_pool,
            tc.tile_pool(name="exp", bufs=2) as ex_pool,
            tc.tile_pool(name="vp", bufs=2 * CPB + 2) as v_pool,
            tc.tile_pool(name="ep", bufs=3) as e_pool,
            tc.tile_pool(name="prp", bufs=3) as pr_pool,
            tc.tile_pool(name="hp", bufs=3) as h_pool,
            tc.tile_pool(name="smp", bufs=4) as sm_pool,
            tc.tile_pool(name="obp", bufs=3) as ob_pool,
            tc.tile_pool(name="pse", bufs=4, space="PSUM") as pse_pool,
            tc.tile_pool(name="psg", bufs=3, space="PSUM") as psg_pool,
        ):
            # ---- constants
            ws0 = cpool.tile([128, 256], F16, tag="ws0")
            ws1 = cpool.tile([128, 256], F16, tag="ws1")
            wd0 = cpool.tile([128, 256], F16, tag="wd0")
            wd1 = cpool.tile([128, 256], F16, tag="wd1")
            nc.sync.dma_start(out=ws0[:], in_=wsrc_d[0])
            nc.sync.dma_start(out=ws1[:], in_=wsrc_d[1])
            nc.sync.dma_start(out=wd0[:], in_=wdst_d[0])
            nc.sync.dma_start(out=wd1[:], in_=wdst_d[1])
            attn_f = cpool.tile([128, 256], F32, tag="attnf")
            nc.gpsimd.dma_start(out=attn_f[:], in_=attn_d[:].to_broadcast((128, 256)))
            attn_b = cpool.tile([128, 256], BF16, tag="attnb")
            nc.vector.tensor_copy(attn_b[:], attn_f[:])
            iota_i = cpool.tile([128, 128], I32, tag="iotai")
            nc.gpsimd.iota(iota_i[:], [[1, 128]], channel_multiplier=0)
            iota_f = cpool.tile([128, 128], F32, tag="iotaf")
            nc.vector.tensor_copy(iota_f[:], iota_i[:])
            if has_bias:
                ones1 = cpool.tile([1, 128], F16, tag="ones1")
                nc.gpsimd.memset(ones1[:], 1.0)
                bs_sb = cpool.tile([1, 256], F16, tag="bs")
                bd_sb = cpool.tile([1, 256], F16, tag="bd")
                nc.sync.dma_start(out=bs_sb[:], in_=bsrc_d[:])
                nc.sync.dma_start(out=bd_sb[:], in_=bdst_d[:])

            for b in range(nb):
                fs0 = fs_pool.tile([128, SC], F16, tag="fs0")
                fs1 = fs_pool.tile([128, SC], F16, tag="fs1")
                fd0 = fd_pool.tile([128, SC], F16, tag="fd0")
                fd1 = fd_pool.tile([128, SC], F16, tag="fd1")
                nc.sync.dma_start(out=fs0[:], in_=fsT_d[0, :, b * SC:(b + 1) * SC])
                nc.sync.dma_start(out=fs1[:], in_=fsT_d[1, :, b * SC:(b + 1) * SC])
                nc.sync.dma_start(out=fd0[:], in_=fdT_d[0, :, b * SC:(b + 1) * SC])
                nc.sync.dma_start(out=fd1[:], in_=fdT_d[1, :, b * SC:(b + 1) * SC])
                rk = rk_pool.tile([128, CPB], F32, tag="rk")
                nc.sync.dma_start(out=rk[:], in_=rankT_d[:, b * CPB:(b + 1) * CPB])
                sc_col = sc_pool.tile([128, CPB], F32, tag="sc")
                vs = []
                for j in range(CPB):
                    s0, s1 = j * CHUNK, (j + 1) * CHUNK
                    pe = pse_pool.tile([128, 256], F32, tag="pe")
                    nc.tensor.matmul(out=pe[:], lhsT=fs0[:, s0:s1], rhs=ws0[:],
                                     start=True, stop=False)
                    nc.tensor.matmul(out=pe[:], lhsT=fs1[:, s0:s1], rhs=ws1[:],
                                     start=False, stop=False)
                    if has_bias:
                        nc.tensor.matmul(out=pe[:], lhsT=ones1[:], rhs=bs_sb[:],
                                         start=False, stop=False)
                    v = v_pool.tile([128, 257], F32, tag="v")
                    nc.gpsimd.memset(v[:, 0:1], 1.0)
                    nc.vector.tensor_copy(v[:, 1:257], pe[:])
                    nc.tensor.matmul(out=pe[:], lhsT=fd0[:, s0:s1], rhs=wd0[:],
                                     start=False, stop=False)
                    nc.tensor.matmul(out=pe[:], lhsT=fd1[:, s0:s1], rhs=wd1[:],
                                     start=False, stop=not has_bias)
                    if has_bias:
                        nc.tensor.matmul(out=pe[:], lhsT=ones1[:], rhs=bd_sb[:],
                                         start=False, stop=True)
                    e_b = e_pool.tile([128, 256], BF16, tag="eb")
                    nc.scalar.activation(e_b[:], pe[:], AF.Prelu, alpha=NEG_SLOPE)
                    pr = pr_pool.tile([128, 256], BF16, tag="pr")
                    nc.gpsimd.tensor_tensor(out=pr[:], in0=e_b[:], in1=attn_b[:],
                                            op=ALU.mult)
                    nc.vector.tensor_reduce(out=sc_col[:, j:j + 1], in_=pr[:],
                                            axis=AX.X, op=ALU.add)
                    vs.append(v)
                exv = ex_pool.tile([128, CPB], F32, tag="ex")
                nc.scalar.activation(exv[:], sc_col[:], AF.Exp)
                for gl in range(GPB):
                    pg = psg_pool.tile([128, 257], F32, tag="pg")
                    for m in range(2):
                        j = gl * 2 + m
                        hx = h_pool.tile([128, 128], F32, tag="hx")
                        nc.gpsimd.tensor_scalar(
                            out=hx[:], in0=iota_f[:], scalar1=rk[:, j:j + 1],
                            scalar2=exv[:, j:j + 1],
                            op0=ALU.is_equal, op1=ALU.mult)
                        nc.tensor.matmul(out=pg[:], lhsT=hx[:], rhs=vs[j][:],
                                         start=(m == 0), stop=(m == 1))
                    ssum = sm_pool.tile([128, 1], F32, tag="ssum")
                    nc.vector.tensor_scalar_max(out=ssum[:], in0=pg[:, 0:1],
                                                scalar1=1e-30)
                    rcp = sm_pool.tile([128, 1], F32, tag="rcp")
                    nc.vector.reciprocal(rcp[:], ssum[:])
                    ob = ob_pool.tile([128, 256], F32, tag="ob")
                    nc.vector.tensor_scalar(out=ob[:], in0=pg[:, 1:257],
                                            scalar1=rcp[:, 0:1], scalar2=None,
                                            op0=ALU.mult)
                    g = b * GPB + gl
                    nc.sync.dma_start(out=dense_d[g * 128:(g + 1) * 128, :],
                                      in_=ob[:])
    nc.compile()
    return nc


# ------------------------------------------------------------------- kernel

def kernel(feat, W_src, b_src, W_dst, b_dst, attn, src, dst, _trace=False):
    global LAST_RESULTS, LAST_NC, LAST_IN_MAPS
    feat = np.asarray(feat, np.float32)
    n_nodes, d_in = feat.shape
    d_out = W_src.shape[1]
    assert d_in == 256 and d_out == 256, "kernel is specialized to D=256"

    p = _prepare(feat, np.asarray(src), np.asarray(dst))
    g_pc, e_slots = p["g_pc"], p["e_slots"]

    has_bias = bool(np.any(b_src) or np.any(b_dst))
    nc = _build_program(g_pc, has_bias)

    feat16 = feat.astype(np.float16)
    wsrc16 = np.ascontiguousarray(
        np.asarray(W_src, np.float32).astype(np.float16).reshape(2, 128, 256))
    wdst16 = np.ascontiguousarray(
        np.asarray(W_dst, np.float32).astype(np.float16).reshape(2, 128, 256))
    attn_in = np.ascontiguousarray(np.asarray(attn, np.float32).reshape(1, 256))

    in_maps = []
    for c in range(N_CORES):
        sl = slice(c * e_slots, (c + 1) * e_slots)
        fs = feat16[p["slot_src"][sl]]          # [e_slots, 256] f16
        fd = feat16[p["slot_dst"][sl]]
        fsT = np.ascontiguousarray(fs.T).reshape(2, 128, e_slots)
        fdT = np.ascontiguousarray(fd.T).reshape(2, 128, e_slots)
        rankT = np.ascontiguousarray(
            p["slot_rank"][sl].reshape(g_pc * 2, 128).T)
        m = {"fsT": fsT, "fdT": fdT, "rankT": rankT,
             "wsrc": wsrc16, "wdst": wdst16, "attn": attn_in}
        if has_bias:
            m["bsrc"] = np.asarray(b_src, np.float32).astype(np.float16).reshape(1, 256)
            m["bdst"] = np.asarray(b_dst, np.float32).astype(np.float16).reshape(1, 256)
        in_maps.append(m)

    res = run_bass_kernel_spmd(nc, in_maps, core_ids=list(range(N_CORES)),
                               trace=_trace)
    LAST_RESULTS, LAST_NC, LAST_IN_MAPS = res, nc, in_maps

    out = np.zeros((n_nodes, 256), np.float32)
    run_core, run_pos, run_node = p["run_core"], p["run_pos"], p["run_node"]
    for c in range(N_CORES):
        dense = res.results[c]["dense"]
        mask = run_core == c
        if not mask.any():
            continue
        if p["need_accum"]:
            np.add.at(out, run_node[mask], dense[run_pos[mask]])
        else:
            out[run_node[mask]] = dense[run_pos[mask]]
    return out


# revision 9
# speedup vs baseline: 1.0207x; 1.0207x over previous
"""GATv2Conv forward on 8 Trainium2 NeuronCores (Bass/Tile).

Strategy
--------
Edges are sorted by destination node and packed into "groups" of at most
S=256 edge slots / at most 128 distinct destinations, such that every
destination's edge run lies entirely inside one group.  Groups are split
evenly across the 8 cores, so all softmax segments and output rows are
core-local: no collectives are needed.

Per core (SPMD, identical program; per-core data differs):
  - el_e = feat[src]@W_src, er_e = feat[dst]@W_dst as dense fp16 GEMMs over
    host-pre-gathered edge features (128-edge chunks, K=256 split in two).
  - score = (prelu(el+er, 0.2) * attn).sum(-1)  (ACT Prelu + mult + reduce)
  - ex = exp(score)  (batched over 8 chunks per ACT op)
  - per group: psum[rank, :] += Hx^T @ [1 | el]  where Hx[e,k] =
    (rank_e==k)*ex_e  (one-hot matmul scatter-add; ssum lands in col 0)
  - out rows = psum[:,1:257] * 1/max(ssum,tiny)  -> dense per-group rows.

Host unshards by scattering dense group rows to their global node ids
(pure index plumbing; all arithmetic happens on-device).
"""

import math
import numpy as np

import concourse.bass as bass
import concourse.mybir as mybir
import concourse.tile as tile
from concourse import bacc
from concourse.bass_utils import run_bass_kernel_spmd

F32 = mybir.dt.float32
F16 = mybir.dt.float16
BF16 = mybir.dt.bfloat16
I32 = mybir.dt.int32
AF = mybir.ActivationFunctionType
ALU = mybir.AluOpType
AX = mybir.AxisListType

N_CORES = 8
S = 256            # edge slots per group (2 chunks of 128)
CHUNK = 128
GPB = 4            # groups per batch (exp batched over 2*GPB chunks)
CPB = 2 * GPB      # chunks per batch
SC = S * GPB       # edge slots per batch
NEG_SLOPE = 0.2

LAST_RESULTS = None  # BassKernelResults of the most recent run (for test.py)
LAST_NC = None       # compiled Bacc program of the most recent run
LAST_IN_MAPS = None  # per-core input dicts of the most recent run


# ----------------------------------------------------------------- host prep

def _pack_runs(counts, max_slots=S, max_nodes=128):
    """Greedily pack whole runs (same-dst edge blocks) into groups."""
    n = len(counts)
    grp = np.empty(n, np.int64)
    rank = np.empty(n, np.int64)
    g = used = nodes = 0
    for i in range(n):
        c = counts[i]
        if used + c > max_slots or nodes >= max_nodes:
            g += 1
            used = 0
            nodes = 0
        grp[i] = g
        rank[i] = nodes
        used += c
        nodes += 1
    return grp, rank, g + 1


def _prepare(feat, src, dst):
    """Sort edges by dst, pack into groups, build per-core input arrays."""
    E = dst.shape[0]
    order = np.argsort(dst, kind="stable")
    sd = dst[order].astype(np.int64)
    ss = src[order].astype(np.int64)

    uniq, counts = np.unique(sd, return_counts=True)
    # split pathological runs longer than S so packing can't fail
    need_accum = bool((counts > S).any())
    if need_accum:
        new_uniq, new_counts = [], []
        for u, c in zip(uniq, counts):
            while c > S:
                new_uniq.append(u)
                new_counts.append(S)
                c -= S
            new_uniq.append(u)
            new_counts.append(c)
        uniq = np.array(new_uniq, np.int64)
        counts = np.array(new_counts, np.int64)

    grp, rank, g_tot = _pack_runs(counts)
    n_runs = len(counts)

    starts = np.zeros(n_runs, np.int64)
    np.cumsum(counts[:-1], out=starts[1:])
    grp_first_run = np.searchsorted(grp, np.arange(g_tot))
    grp_start_edge = starts[grp_first_run]

    run_of_edge = np.repeat(np.arange(n_runs), counts)
    e_grp = grp[run_of_edge]
    e_rank = rank[run_of_edge]
    e_slot = e_grp * S + np.arange(E) - grp_start_edge[e_grp]

    g_pc = math.ceil(g_tot / N_CORES)
    g_pc = math.ceil(g_pc / GPB) * GPB          # multiple of GPB per core
    e_slots = g_pc * S
    total_slots = N_CORES * e_slots

    slot_src = np.zeros(total_slots, np.int64)
    slot_dst = np.zeros(total_slots, np.int64)
    slot_rank = np.full(total_slots, -1.0, np.float32)
    slot_src[e_slot] = ss
    slot_dst[e_slot] = sd
    slot_rank[e_slot] = e_rank

    # unshard info per run
    run_core = grp // g_pc
    run_pos = (grp % g_pc) * 128 + rank        # row in the core's dense output

    return dict(
        g_pc=g_pc, e_slots=e_slots, need_accum=need_accum,
        slot_src=slot_src, slot_dst=slot_dst, slot_rank=slot_rank,
        run_core=run_core, run_pos=run_pos, run_node=uniq,
    )


# ------------------------------------------------------------ device program

def _build_program(n_g, has_bias, repeat=1):
    nc_chunks = n_g * 2
    e_slots = n_g * S
    nb = n_g // GPB
    nc = bacc.Bacc("TRN2", target_bir_lowering=False, debug=False,
                   num_devices=N_CORES)
    fsT_d = nc.dram_tensor("fsT", [2, 128, e_slots], F16, kind="ExternalInput").ap()
    fdT_d = nc.dram_tensor("fdT", [2, 128, e_slots], F16, kind="ExternalInput").ap()
    rankT_d = nc.dram_tensor("rankT", [128, nc_chunks], F32, kind="ExternalInput").ap()
    wsrc_d = nc.dram_tensor("wsrc", [2, 128, 256], F16, kind="ExternalInput").ap()
    wdst_d = nc.dram_tensor("wdst", [2, 128, 256], F16, kind="ExternalInput").ap()
    attn_d = nc.dram_tensor("attn", [1, 256], F32, kind="ExternalInput").ap()
    if has_bias:
        bsrc_d = nc.dram_tensor("bsrc", [1, 256], F16, kind="ExternalInput").ap()
        bdst_d = nc.dram_tensor("bdst", [1, 256], F16, kind="ExternalInput").ap()
    dense_d = nc.dram_tensor("dense", [n_g * 128, 256], F32,
                             kind="ExternalOutput").ap()

    with tile.TileContext(nc) as tc:
        with (
            tc.tile_pool(name="const", bufs=1) as cpool,
            tc.tile_pool(name="fsp", bufs=2) as fs_pool,
            tc.tile_pool(name="fdp", bufs=2) as fd_pool,
            tc.tile_pool(name="rkp", bufs=2) as rk_pool,
            tc.tile_pool(name="scp", bufs=2) as sc_pool,
            tc.tile_pool(name="exp", bufs=2) as ex_pool,
            tc.tile_pool(name="vp", bufs=2 * CPB + 2) as v_pool,
            tc.tile_pool(name="ep", bufs=3) as e_pool,
            tc.tile_pool(name="prp", bufs=3) as pr_pool,
            tc.tile_pool(name="hp", bufs=3) as h_pool,
            tc.tile_pool(name="smp", bufs=4) as sm_pool,
            tc.tile_pool(name="obp", bufs=3) as ob_pool,
            tc.tile_pool(name="pse", bufs=4, space="PSUM") as pse_pool,
            tc.tile_pool(name="psg", bufs=3, space="PSUM") as psg_pool,
        ):
            # ---- constants
            ws0 = cpool.tile([128, 256], F16, tag="ws0")
            ws1 = cpool.tile([128, 256], F16, tag="ws1")
            wd0 = cpool.tile([128, 256], F16, tag="wd0")
            wd1 = cpool.tile([128, 256], F16, tag="wd1")
            nc.sync.dma_start(out=ws0[:], in_=wsrc_d[0])
            nc.sync.dma_start(out=ws1[:], in_=wsrc_d[1])
            nc.sync.dma_start(out=wd0[:], in_=wdst_d[0])
            nc.sync.dma_start(out=wd1[:], in_=wdst_d[1])
            attn_f = cpool.tile([128, 256], F32, tag="attnf")
            nc.gpsimd.dma_start(out=attn_f[:], in_=attn_d[:].to_broadcast((128, 256)))
            attn_b = cpool.tile([128, 256], BF16, tag="attnb")
            nc.vector.tensor_copy(attn_b[:], attn_f[:])
            iota_i = cpool.tile([128, 128], I32, tag="iotai")
            nc.gpsimd.iota(iota_i[:], [[1, 128]], channel_multiplier=0)
            iota_f = cpool.tile([128, 128], F32, tag="iotaf")
            nc.vector.tensor_copy(iota_f[:], iota_i[:])
            if has_bias:
                ones1 = cpool.tile([1, 128], F16, tag="ones1")
                nc.gpsimd.memset(ones1[:], 1.0)
                bs_sb = cpool.tile([1, 256], F16, tag="bs")
                bd_sb = cpool.tile([1, 256], F16, tag="bd")
                nc.sync.dma_start(out=bs_sb[:], in_=bsrc_d[:])
                nc.sync.dma_start(out=bd_sb[:], in_=bdst_d[:])

            import contextlib
            _rep = contextlib.ExitStack()
            if repeat > 1:
                _rep.enter_context(tc.For_i(0, repeat, 1))
            for b in range(nb):
                fs0 = fs_pool.tile([128, SC], F16, tag="fs0")
                fs1 = fs_pool.tile([128, SC], F16, tag="fs1")
                fd0 = fd_pool.tile([128, SC], F16, tag="fd0")
                fd1 = fd_pool.tile([128, SC], F16, tag="fd1")
                nc.sync.dma_start(out=fs0[:], in_=fsT_d[0, :, b * SC:(b + 1) * SC])
                nc.sync.dma_start(out=fs1[:], in_=fsT_d[1, :, b * SC:(b + 1) * SC])
                nc.sync.dma_start(out=fd0[:], in_=fdT_d[0, :, b * SC:(b + 1) * SC])
                nc.sync.dma_start(out=fd1[:], in_=fdT_d[1, :, b * SC:(b + 1) * SC])
                rk = rk_pool.tile([128, CPB], F32, tag="rk")
                nc.sync.dma_start(out=rk[:], in_=rankT_d[:, b * CPB:(b + 1) * CPB])
                sc_col = sc_pool.tile([128, CPB], F32, tag="sc")
                vs = []
                for j in range(CPB):
                    s0, s1 = j * CHUNK, (j + 1) * CHUNK
                    pe = pse_pool.tile([128, 256], F32, tag="pe")
                    nc.tensor.matmul(out=pe[:], lhsT=fs0[:, s0:s1], rhs=ws0[:],
                                     start=True, stop=False)
                    nc.tensor.matmul(out=pe[:], lhsT=fs1[:, s0:s1], rhs=ws1[:],
                                     start=False, stop=False)
                    if has_bias:
                        nc.tensor.matmul(out=pe[:], lhsT=ones1[:], rhs=bs_sb[:],
                                         start=False, stop=False)
                    v = v_pool.tile([128, 257], F32, tag="v")
                    nc.gpsimd.memset(v[:, 0:1], 1.0)
                    nc.vector.tensor_copy(v[:, 1:257], pe[:])
                    nc.tensor.matmul(out=pe[:], lhsT=fd0[:, s0:s1], rhs=wd0[:],
                                     start=False, stop=False)
                    nc.tensor.matmul(out=pe[:], lhsT=fd1[:, s0:s1], rhs=wd1[:],
                                     start=False, stop=not has_bias)
                    if has_bias:
                        nc.tensor.matmul(out=pe[:], lhsT=ones1[:], rhs=bd_sb[:],
                                         start=False, stop=True)
                    e_b = e_pool.tile([128, 256], BF16, tag="eb")
                    nc.scalar.activation(e_b[:], pe[:], AF.Prelu, alpha=NEG_SLOPE)
                    pr = pr_pool.tile([128, 256], BF16, tag="pr")
                    nc.gpsimd.tensor_tensor(out=pr[:], in0=e_b[:], in1=attn_b[:],
                                            op=ALU.mult)
                    nc.vector.tensor_reduce(out=sc_col[:, j:j + 1], in_=pr[:],
                                            axis=AX.X, op=ALU.add)
                    vs.append(v)
                exv = ex_pool.tile([128, CPB], F32, tag="ex")
                nc.scalar.activation(exv[:], sc_col[:], AF.Exp)
                for gl in range(GPB):
                    pg = psg_pool.tile([128, 257], F32, tag="pg")
                    for m in range(2):
                        j = gl * 2 + m
                        hx = h_pool.tile([128, 128], F32, tag="hx")
                        nc.gpsimd.tensor_scalar(
                            out=hx[:], in0=iota_f[:], scalar1=rk[:, j:j + 1],
                            scalar2=exv[:, j:j + 1],
                            op0=ALU.is_equal, op1=ALU.mult)
                        nc.tensor.matmul(out=pg[:], lhsT=hx[:], rhs=vs[j][:],
                                         start=(m == 0), stop=(m == 1))
                    ssum = sm_pool.tile([128, 1], F32, tag="ssum")
                    nc.vector.tensor_scalar_max(out=ssum[:], in0=pg[:, 0:1],
                                                scalar1=1e-30)
                    rcp = sm_pool.tile([128, 1], F32, tag="rcp")
                    nc.vector.reciprocal(rcp[:], ssum[:])
                    ob = ob_pool.tile([128, 256], F32, tag="ob")
                    nc.vector.tensor_scalar(out=ob[:], in0=pg[:, 1:257],
                                            scalar1=rcp[:, 0:1], scalar2=None,
                                            op0=ALU.mult)
                    g = b * GPB + gl
                    nc.sync.dma_start(out=dense_d[g * 128:(g + 1) * 128, :],
                                      in_=ob[:])
            _rep.close()
    nc.compile()
    return nc


# ------------------------------------------------------------------- kernel

def kernel(feat, W_src, b_src, W_dst, b_dst, attn, src, dst, _trace=False):
    global LAST_RESULTS, LAST_NC, LAST_IN_MAPS
    feat = np.asarray(feat, np.float32)
    n_nodes, d_in = feat.shape
    d_out = W_src.shape[1]
    assert d_in == 256 and d_out == 256, "kernel is specialized to D=256"

    p = _prepare(feat, np.asarray(src), np.asarray(dst))
    g_pc, e_slots = p["g_pc"], p["e_slots"]

    has_bias = bool(np.any(b_src) or np.any(b_dst))
    nc = _build_program(g_pc, has_bias)

    feat16 = feat.astype(np.float16)
    wsrc16 = np.ascontiguousarray(
        np.asarray(W_src, np.float32).astype(np.float16).reshape(2, 128, 256))
    wdst16 = np.ascontiguousarray(
        np.asarray(W_dst, np.float32).astype(np.float16).reshape(2, 128, 256))
    attn_in = np.ascontiguousarray(np.asarray(attn, np.float32).reshape(1, 256))

    in_maps = []
    for c in range(N_CORES):
        sl = slice(c * e_slots, (c + 1) * e_slots)
        fs = feat16[p["slot_src"][sl]]          # [e_slots, 256] f16
        fd = feat16[p["slot_dst"][sl]]
        fsT = np.ascontiguousarray(fs.T).reshape(2, 128, e_slots)
        fdT = np.ascontiguousarray(fd.T).reshape(2, 128, e_slots)
        rankT = np.ascontiguousarray(
            p["slot_rank"][sl].reshape(g_pc * 2, 128).T)
        m = {"fsT": fsT, "fdT": fdT, "rankT": rankT,
             "wsrc": wsrc16, "wdst": wdst16, "attn": attn_in}
        if has_bias:
            m["bsrc"] = np.asarray(b_src, np.float32).astype(np.float16).reshape(1, 256)
            m["bdst"] = np.asarray(b_dst, np.float32).astype(np.float16).reshape(1, 256)
        in_maps.append(m)

    res = run_bass_kernel_spmd(nc, in_maps, core_ids=list(range(N_CORES)),
                               trace=_trace)
    LAST_RESULTS, LAST_NC, LAST_IN_MAPS = res, nc, in_maps

    out = np.zeros((n_nodes, 256), np.float32)
    run_core, run_pos, run_node = p["run_core"], p["run_pos"], p["run_node"]
    for c in range(N_CORES):
        dense = res.results[c]["dense"]
        mask = run_core == c
        if not mask.any():
            continue
        if p["need_accum"]:
            np.add.at(out, run_node[mask], dense[run_pos[mask]])
        else:
            out[run_node[mask]] = dense[run_pos[mask]]
    return out
